# revision 1
# baseline (speedup 1.0000x reference)
"""Causal multi-head attention (B=2, S=2048, D=1024, H=16) on 8 TRN2 NeuronCores.

Sharding: batch*heads across cores. Core c handles batch c//4 and the 4 heads
g*4..g*4+3 where g = c%4. Weights are sliced per core (Megatron-style column
split of Wq/Wk/Wv, row split of Wo); each core produces a partial projected
output [D, S] (transposed) and the host sums the 4 partials per batch.

Everything on-chip is kept transposed ([feature, seq]) so no transposes are
ever needed:
  qT/kT = wq/wk^T @ xT            (PE, contraction over D)
  v     = x @ Wv^T                (s on partitions, + ones column appended)
  sT    = k @ qT  [s_k=128, s_q]  (PE, contraction over dh=64, 2 heads packed
                                   via base-partition 0/64 row groups)
  eT    = exp(sT/8) * causal_mask (ACT + GPSIMD; mask is 0/1, exact)
  av    = v_aug^T @ eT -> [65, s_q]  row 64 = softmax denominator
  outT  = av[0:64] * (1/denom broadcast via DRAM-bounce DMA)
  partialT = wo^T-chunks @ outT   (PE, contraction over the 256 head dims)

All matmuls run as float32r (tf32-like, 1 col/cycle vs 4 for fp32).

Scheduling: engines execute their instruction streams in emission order, so
the kernel is emitted as ONE interleaved stream: attention chunks (which are
latency-bound on the PE->ACT->GPSIMD->PE chain) are interspersed with
"filler" QKV / Wo accumulation groups that keep the PE busy while exp/mask
run, and AV matmuls trail their scores by one chunk (AV lag). Causality
guarantees attention(t) only needs k/v chunks from tiles <= t.
"""

from collections import deque

import numpy as np

import concourse.bass as bass
import concourse.mybir as mybir
import concourse.tile as tile
from concourse import bacc
from concourse.bass_utils import run_bass_kernel_spmd

B = 2
S = 2048
D = 1024
H = 16
DH = 64
N_CORES = 8
HG = H // 4  # 4 heads per core
GM = 4 * DH  # 256 head dims per core
FP32 = mybir.dt.float32
FP32R = mybir.dt.float32r

S_TILE = 512  # q-tile width (PSUM bank)
N_ST = S // S_TILE  # 4
KC = 128  # k-chunk (partition dim of scoresT)
N_KC = S // KC  # 16
N_DC = D // 128  # 8 d-chunks
AV_LAG = 4  # chunks between scores and their AV matmuls (hides exp+mask latency)


def build_program():
    nc = bacc.Bacc("TRN2", target_bir_lowering=False, debug=False)

    xT = nc.dram_tensor("xT", [D, S], FP32R, kind="ExternalInput")
    wq = nc.dram_tensor("wq", [D, GM], FP32R, kind="ExternalInput")
    wk = nc.dram_tensor("wk", [D, GM], FP32R, kind="ExternalInput")
    wv = nc.dram_tensor("wv", [D, GM], FP32R, kind="ExternalInput")
    wo = nc.dram_tensor("wo", [GM, D], FP32R, kind="ExternalInput")
    # mask[i, j, 1024] = causal pattern j duplicated for the 2 packed heads
    mask = nc.dram_tensor("mask", [KC, 4 * 2 * S_TILE], FP32, kind="ExternalInput")
    outT = nc.dram_tensor("outT", [D, S], FP32, kind="ExternalOutput")

    with tile.TileContext(nc) as tc:
        with (
            tc.tile_pool(name="persist", bufs=1) as persist,
            tc.tile_pool(name="xb", bufs=12) as xb_pool,
            tc.tile_pool(name="exp", bufs=8) as exp_pool,
            tc.tile_pool(name="small", bufs=4) as small_pool,
            tc.tile_pool(name="outsb", bufs=3) as out_pool,
            tc.tile_pool(name="dram", bufs=4, space="DRAM") as dram_pool,
            tc.tile_pool(name="mm", bufs=2, space="PSUM") as mm_pool,
            tc.tile_pool(name="scores", bufs=2, space="PSUM") as sc_pool,
            tc.tile_pool(name="av", bufs=2, space="PSUM") as av_pool,
        ):
            # ---- persistent SBUF tensors ----
            wo_sb = persist.tile([128, 2, D], FP32R, tag="wo")
            mask_sb = persist.tile([KC, 4, 2 * S_TILE], FP32, tag="mask")
            ones_col = persist.tile([128, 1], FP32, tag="ones")
            ones_row = persist.tile([1, DH], FP32R, tag="onesr")
            nc.vector.memset(ones_col[:, :], 1.0)
            w_sb = {}
            for name, w in (("q", wq), ("k", wk), ("v", wv)):
                w_sb[name] = persist.tile(
                    [128, N_DC, GM], FP32R, tag=f"w{name}", name=f"w{name}sb"
                )
            # DMA order matters: the k/q weights and tile-0 x blocks gate the
            # first matmuls, so they go first (chunk-interleaved with the x
            # blocks below); mask/wo aren't needed until much later.
            wk_r = wk.rearrange("(c p) m -> p c m", p=128)
            wq_r = wq.rearrange("(c p) m -> p c m", p=128)
            nc.vector.tensor_copy(ones_row[:, :], ones_col[0:1, 0:1].broadcast_to((1, DH)))

            qT = {}  # (u, t) -> [128, 512]   2 heads stacked (rows 0-63 / 64-127)
            kT = {}
            vt = {}  # c16 -> [128, HG, 65]   v chunk with ones col per head
            oT = {}  # (u, t) -> [128, 512]
            for t in range(N_ST):
                for u in range(2):
                    qT[(u, t)] = persist.tile(
                        [128, S_TILE], FP32R, tag=f"qT{u}{t}", name=f"qT{u}{t}"
                    )
                    kT[(u, t)] = persist.tile(
                        [128, S_TILE], FP32R, tag=f"kT{u}{t}", name=f"kT{u}{t}"
                    )
                    oT[(u, t)] = persist.tile(
                        [128, S_TILE], FP32R, tag=f"oT{u}{t}", name=f"oT{u}{t}"
                    )
            for c16 in range(N_KC):
                vt[c16] = persist.tile(
                    [128, HG, DH + 1], FP32R, tag=f"v{c16}", name=f"v{c16}"
                )

            # xT loaded as [d-chunk, s-tile] blocks so compute starts early
            xb = {}

            def load_xb(t):
                for c in range(N_DC):
                    blk = xb_pool.tile([128, S_TILE], FP32R, tag="xb", name=f"xb{c}_{t}")
                    nc.sync.dma_start(
                        blk[:, :],
                        xT[c * 128 : (c + 1) * 128, t * S_TILE : (t + 1) * S_TILE],
                    )
                    xb[(c, t)] = blk

            # chunk-interleaved: first accumulation matmul can start after
            # ~0.5MB instead of after all weights+x.
            for c in range(N_DC):
                nc.sync.dma_start(w_sb["k"][:, c, :], wk_r[:, c, :])
                nc.sync.dma_start(w_sb["q"][:, c, :], wq_r[:, c, :])
                blk = xb_pool.tile([128, S_TILE], FP32R, tag="xb", name=f"xb{c}_0")
                nc.sync.dma_start(
                    blk[:, :], xT[c * 128 : (c + 1) * 128, 0:S_TILE]
                )
                xb[(c, 0)] = blk
            nc.sync.dma_start(
                w_sb["v"][:, :, :], wv.rearrange("(c p) m -> p c m", p=128)
            )
            for _t in range(1, N_ST):
                load_xb(_t)
            nc.sync.dma_start(mask_sb[:, :, :], mask.rearrange("p (j n) -> p j n", j=4))
            nc.sync.dma_start(wo_sb[:, :, :], wo.rearrange("(u p) d -> p u d", p=128))

            # ---- emission thunks ----
            def emit_qk_group(name, u, t):
                dst = kT if name == "k" else qT
                ps = mm_pool.tile([128, S_TILE], FP32, tag="mm", name=f"ps{name}{u}{t}")
                for c in range(N_DC):
                    nc.tensor.matmul(
                        ps[:, :],
                        lhsT=w_sb[name][:, c, u * 128 : (u + 1) * 128],
                        rhs=xb[(c, t)][:, :],
                        start=(c == 0),
                        stop=(c == N_DC - 1),
                    )
                nc.vector.tensor_copy(dst[(u, t)][:, :], ps[:, :])

            def emit_v_group(t, s4):
                c16 = 4 * t + s4
                ps = mm_pool.tile([128, GM], FP32, tag="mm", name=f"psv{c16}")
                for c in range(N_DC):
                    nc.tensor.matmul(
                        ps[:, :],
                        lhsT=xb[(c, t)][:, s4 * 128 : (s4 + 1) * 128],
                        rhs=w_sb["v"][:, c, :],
                        start=(c == 0),
                        stop=(c == N_DC - 1),
                    )
                nc.vector.tensor_copy(
                    vt[c16][:, :, 0:DH], ps.rearrange("p (h d) -> p h d", h=HG)
                )
                nc.gpsimd.tensor_copy(
                    vt[c16][:, :, DH : DH + 1],
                    ones_col[:, 0:1].broadcast_to((128, HG, 1)),
                )

            def emit_wo_group(t, dc):
                po = mm_pool.tile([128, S_TILE], FP32, tag="mm", name=f"po{t}{dc}")
                for u in range(2):
                    nc.tensor.matmul(
                        po[:, :],
                        lhsT=wo_sb[:, u, dc * 128 : (dc + 1) * 128],
                        rhs=oT[(u, t)][:, :],
                        start=(u == 0),
                        stop=(u == 1),
                    )
                ob = out_pool.tile([128, S_TILE], FP32, tag="ob")
                nc.vector.tensor_copy(ob[:, :], po[:, :])
                nc.sync.dma_start(
                    outT[dc * 128 : (dc + 1) * 128, t * S_TILE : (t + 1) * S_TILE],
                    ob[:, :],
                )

            # filler queue: (tile, thunk). pump() emits fillers between attention
            # chunks to keep the PE dense while exp/mask latency elapses.
            fillers = deque()
            for t in range(N_ST):
                if t == 0:
                    pass  # t=0 prologue emitted directly below
                else:
                    for u in range(2):
                        fillers.append((t, lambda u=u, t=t: emit_qk_group("k", u, t)))
                        fillers.append((t, lambda u=u, t=t: emit_qk_group("q", u, t)))
                    for s4 in range(4):
                        fillers.append((t, lambda t=t, s4=s4: emit_v_group(t, s4)))

            def pump(n):
                for _ in range(n):
                    if not fillers:
                        return
                    _, thunk = fillers.popleft()
                    thunk()

            def flush_through_tile(t):
                while fillers and fillers[0][0] <= t:
                    _, thunk = fillers.popleft()
                    thunk()

            # prologue: only what attention(0, hp=0) needs; the rest of
            # tile-0 QKV becomes filler work
            emit_qk_group("k", 0, 0)
            emit_qk_group("q", 0, 0)
            for s4 in range(4):
                emit_v_group(0, s4)
            fillers.appendleft((0, lambda: emit_qk_group("q", 1, 0)))
            fillers.appendleft((0, lambda: emit_qk_group("k", 1, 0)))

            for t in range(N_ST):
                nch = 4 * t + 4
                for hp in range(2):
                    u = hp
                    if t > 0 or hp > 0:
                        flush_through_tile(t)  # qkv(<=t) must be emitted
                    avs = [
                        av_pool.tile(
                            [DH + 1, S_TILE], FP32, tag="av", name=f"av{t}{hp}{i}"
                        )
                        for i in range(2)
                    ]
                    pending_avs = deque()  # AV trails scores by AV_LAG chunks
                    for c in range(nch):
                        # Diagonal chunks only touch q columns >= 128j (causal):
                        # scores / exp / AV all skip the fully-masked prefix.
                        j = c - 4 * t
                        q0 = 128 * j if j >= 0 else 0
                        w = S_TILE - q0
                        sc = sc_pool.tile([128, 2 * S_TILE], FP32, tag="sc")
                        for i in range(2):  # head parity: rows 0-63 / 64-127
                            bp = 64 * i
                            nc.tensor.matmul(
                                sc[:, i * S_TILE + q0 : (i + 1) * S_TILE],
                                lhsT=kT[(u, c // 4)][
                                    bp : bp + DH, (c % 4) * 128 : (c % 4 + 1) * 128
                                ],
                                rhs=qT[(u, t)][bp : bp + DH, q0:],
                                start=True,
                                stop=True,
                            )
                        ex = exp_pool.tile([128, 2 * S_TILE], FP32R, tag="ex")
                        exv = ex.rearrange("p (i n) -> p i n", i=2)[:, :, q0:]
                        scv = sc.rearrange("p (i n) -> p i n", i=2)[:, :, q0:]
                        nc.scalar.activation(
                            exv, scv, mybir.ActivationFunctionType.Exp, scale=0.125
                        )
                        if j >= 0:  # mask the 128-wide triangular band
                            for i in range(2):
                                sl = slice(i * S_TILE + q0, i * S_TILE + q0 + 128)
                                nc.gpsimd.tensor_mul(
                                    ex[:, sl], ex[:, sl], mask_sb[:, j, sl]
                                )

                        def emit_av(cc, exx, qq0):
                            for i in range(2):
                                nc.tensor.matmul(
                                    avs[i][:, qq0:],
                                    lhsT=vt[cc][:, 2 * hp + i, :],
                                    rhs=exx[:, i * S_TILE + qq0 : (i + 1) * S_TILE],
                                    start=(cc == 0),
                                    stop=(cc == nch - 1),
                                )

                        pending_avs.append((c, ex, q0))
                        pump(1)
                        if len(pending_avs) > AV_LAG:
                            emit_av(*pending_avs.popleft())
                    while pending_avs:
                        emit_av(*pending_avs.popleft())
                    for i in range(2):
                        bp = 64 * i
                        den = small_pool.tile([1, S_TILE], FP32, tag="den")
                        nc.vector.tensor_copy(den[:, :], avs[i][DH : DH + 1, :])
                        rec32 = small_pool.tile([1, S_TILE], FP32, tag="rec32")
                        nc.vector.reciprocal_approx_fast(rec32[:, :], den[:, :])
                        rec = small_pool.tile([1, S_TILE], FP32R, tag="rec")
                        nc.vector.tensor_copy(rec[:, :], rec32[:, :])
                        bc = mm_pool.tile([DH, S_TILE], FP32, tag="mm", name=f"bc{t}{hp}{i}")
                        nc.tensor.matmul(
                            bc[:, :], lhsT=ones_row[:, :], rhs=rec[:, :],
                            start=True, stop=True,
                        )
                        bc_sb = small_pool.tile([DH, S_TILE], FP32, tag="bcsb")
                        nc.scalar.copy(bc_sb[:, :], bc[:, :])
                        nc.vector.tensor_mul(
                            oT[(u, t)][bp : bp + DH, :], avs[i][0:DH, :], bc_sb[:, :]
                        )
                        pump(2)
                # Wo for this tile becomes filler work for later attention
                for dc in range(N_DC):
                    fillers.append((t, lambda t=t, dc=dc: emit_wo_group(t, dc)))
            while fillers:
                fillers.popleft()[1]()
    nc.compile()
    return nc


_NC_CACHE = None


def _get_program():
    global _NC_CACHE
    if _NC_CACHE is None:
        _NC_CACHE = build_program()
    return _NC_CACHE


def _make_mask():
    # pattern j: mask[i, q] = 1.0 iff (128*j + i) <= q; duplicated for 2 heads
    i = np.arange(KC)[:, None]
    q = np.arange(S_TILE)[None, :]
    blocks = [np.tile((128 * j + i <= q).astype(np.float32), (1, 2)) for j in range(4)]
    return np.concatenate(blocks, axis=1)  # [128, 4*1024]


def _make_in_maps(x, Wq, Wk, Wv, Wo):
    mask = _make_mask()
    xTs = [np.ascontiguousarray(x[b].T) for b in range(B)]
    in_maps = []
    for core in range(N_CORES):
        b, g = divmod(core, HG)
        r0, r1 = g * GM, (g + 1) * GM
        in_maps.append(
            {
                "xT": xTs[b],
                "wq": np.ascontiguousarray(Wq[r0:r1, :].T),
                "wk": np.ascontiguousarray(Wk[r0:r1, :].T),
                "wv": np.ascontiguousarray(Wv[r0:r1, :].T),
                "wo": np.ascontiguousarray(Wo[:, r0:r1].T),
                "mask": mask,
            }
        )
    return in_maps


def kernel(x, Wq, Wk, Wv, Wo, **_unused):
    x = np.asarray(x, dtype=np.float32)
    Wq = np.asarray(Wq, dtype=np.float32)
    Wk = np.asarray(Wk, dtype=np.float32)
    Wv = np.asarray(Wv, dtype=np.float32)
    Wo = np.asarray(Wo, dtype=np.float32)

    nc = _get_program()
    in_maps = _make_in_maps(x, Wq, Wk, Wv, Wo)
    res = run_bass_kernel_spmd(nc, in_maps, core_ids=list(range(N_CORES)))
    out = np.zeros((B, S, D), dtype=np.float64)
    for core in range(N_CORES):
        b = core // HG
        out[b] += res.results[core]["outT"].T.astype(np.float64)
    return out.astype(np.float32)

